# revision 10
# baseline (speedup 1.0000x reference)
"""Trainium2 Bass kernel for nn_CompactControlAttention.

The module's attention is degenerate: softmax over a size-1 axis is exactly
1.0, so queries/keys (Wq, bq, Wk, bk) never affect the output:

    out[b, s, :] = sequence[b, s, :] + p[b, :]
    p = (sum_c controls[c]) @ Wv.T @ Wo.T + C * (bv @ Wo.T + bo)

Sharding (8 cores, no collectives): tensor-parallel over the OUTPUT feature
dim e. Core k computes out[:, :, 256k:256(k+1)], which needs full Wv
(replicated), a 256-column slice of Wo, and the matching slice of
sequence/bo. Weight matrices are shipped pre-transposed ([in, out] layout)
so the contraction dim lands on SBUF partitions without on-device
transposes of the big weights.

Per-core device program:
  cs = sum_c controls[c]                 (DVE tree over 4 SBUF tiles)
  csT = cs.T                             (16 PE transposes, 64x128 blocks)
  v = cs @ Wv.T + C*bv                   (PSUM accum over 16 K-tiles)
  vT = v.T                               (16 PE transposes)
  p = v @ WoT_k + C/8*... (+ bo_k)       (16 K-tiles of 128)
  out = seq_k + broadcast_s(p)           (PE outer-product expand + DVE add)
"""

import numpy as np

import concourse.bass as bass
import concourse.mybir as mybir
import concourse.tile as tile
from concourse import bacc
from concourse.bass_utils import run_bass_kernel_spmd
from concourse.masks import make_identity

N_CORES = 8
D = 2048
B = 64
S = 32
C = 8
EK = D // N_CORES  # 256: output-feature slice per core
ROWS = B * S  # 2048 flattened (b, s) rows
F32 = mybir.dt.float32

_CACHE = {}


def _build_nc(mm_dt=F32):
    nc = bacc.Bacc("TRN2", target_bir_lowering=False, debug=False, num_devices=N_CORES)

    # Per-core inputs.
    seq = nc.dram_tensor("seq", [128, 2 * B * S], F32, kind="ExternalInput")
    ctrl = nc.dram_tensor("ctrl", [C * B, D], F32, kind="ExternalInput")
    wvt = nc.dram_tensor("wvt", [D, D], F32, kind="ExternalInput")  # Wv.T [f, d]
    wot = nc.dram_tensor("wot", [D, EK], F32, kind="ExternalInput")  # Wo.T[:, e_k]
    bv = nc.dram_tensor("bv", [D], F32, kind="ExternalInput")
    bo = nc.dram_tensor("bo", [EK], F32, kind="ExternalInput")
    out = nc.dram_tensor("out", [128, 2 * B * S], F32, kind="ExternalOutput")

    with tile.TileContext(nc) as tc:
        _body(tc, seq, ctrl, wvt, wot, bv, bo, out, mm_dt)
    nc.compile()
    return nc


def _body(tc, seq, ctrl, wvt, wot, bv, bo, out, mm_dt):
    from contextlib import ExitStack

    ctx = ExitStack()
    nc = tc.nc
    P = 128

    consts = ctx.enter_context(tc.tile_pool(name="consts", bufs=1))
    sbuf = ctx.enter_context(tc.tile_pool(name="sbuf", bufs=1))
    wpool = ctx.enter_context(tc.tile_pool(name="wv", bufs=3))
    psum_t = ctx.enter_context(tc.tile_pool(name="psum_t", bufs=2, space="PSUM"))
    psum_v = ctx.enter_context(tc.tile_pool(name="psum_v", bufs=1, space="PSUM"))
    psum_p = ctx.enter_context(tc.tile_pool(name="psum_p", bufs=1, space="PSUM"))

    # --- constants -------------------------------------------------------
    ident = consts.tile([P, P], mm_dt)
    make_identity(nc, ident[:])
    # sel = two stacked 64x64 identities: transpose-matmul against sel folds
    # the last c-parity pair while transposing.
    sel = consts.tile([P, B], mm_dt)
    nc.gpsimd.dma_start(out=sel[0:B, :], in_=ident[0:B, 0:B])
    nc.gpsimd.dma_start(out=sel[B : 2 * B, :], in_=ident[0:B, 0:B])
    ones8 = consts.tile([1, B], mm_dt)  # value C: bias-augment row for MM1
    nc.vector.memset(ones8[:], float(C))
    ones1 = consts.tile([1, B], mm_dt)  # value 1: bias-augment row for MM2
    nc.vector.memset(ones1[:], 1.0)
    bv_sb = consts.tile([1, D], mm_dt)
    nc.sync.dma_start(out=bv_sb[:], in_=bv[None, :])
    bo_sb = consts.tile([1, EK], mm_dt)
    nc.sync.dma_start(out=bo_sb[:], in_=bo[None, :])

    # --- controls: load + fold over C ------------------------------------
    # ctrl is (C*B, D) = (512, 2048) -> one [128, 4*2048] tile, group g holds
    # rows 128g..128g+127.
    ctrl_sb = sbuf.tile([P, 4 * D], F32)
    nc.sync.dma_start(
        out=ctrl_sb[:].rearrange("p (g d) -> p g d", d=D),
        in_=ctrl.rearrange("(g p) d -> p g d", p=P),
    )
    acc = sbuf.tile([P, D], F32)
    nc.vector.tensor_add(acc[:], ctrl_sb[:, 0:D], ctrl_sb[:, D : 2 * D])
    nc.vector.tensor_add(acc[:], acc[:], ctrl_sb[:, 2 * D : 3 * D])
    nc.vector.tensor_add(acc[:], acc[:], ctrl_sb[:, 3 * D : 4 * D])

    # --- csT: fold last c-pair + transpose in one matmul per f-block -----
    # acc row p = (parity, b); acc.T @ sel sums the two parity rows per b.
    cst = sbuf.tile([P, 16 * B], mm_dt)  # block j at cols [64j, 64j+64)
    for j in range(16):
        pt = psum_t.tile([P, B], F32, tag="pt")
        nc.tensor.matmul(
            pt[:], acc[:, j * P : (j + 1) * P], sel[:], start=True, stop=True
        )
        nc.vector.tensor_copy(cst[:, j * B : (j + 1) * B], pt[:])

    # --- MM1: v = cs @ Wv.T + C*bv  (v in 4 PSUM banks of [64, 512]) -----
    pv = [psum_v.tile([B, 512], F32, tag=f"pv{c}", name=f"pv{c}") for c in range(4)]
    for jj in range(8):  # stream Wv.T in 2MB chunks of two 128-row tiles
        wv_sb = wpool.tile([P, 2 * D], mm_dt)
        nc.sync.dma_start(
            out=wv_sb[:].rearrange("p (g d) -> p g d", d=D),
            in_=wvt[jj * 256 : (jj + 1) * 256, :].rearrange("(g p) d -> p g d", p=P),
        )
        for g in range(2):
            j = 2 * jj + g
            for c in range(4):
                nc.tensor.matmul(
                    pv[c][:],
                    cst[:, j * B : (j + 1) * B],
                    wv_sb[:, g * D + c * 512 : g * D + (c + 1) * 512],
                    start=(j == 0),
                    stop=False,
                )
    for c in range(4):  # bias-augment row: += C * bv
        nc.tensor.matmul(
            pv[c][:],
            ones8[:],
            bv_sb[:, c * 512 : (c + 1) * 512],
            start=False,
            stop=True,
        )
    v = sbuf.tile([B, D], mm_dt)
    for c in range(4):
        nc.vector.tensor_copy(v[:, c * 512 : (c + 1) * 512], pv[c][:])

    # --- vT: 16 PE transposes --------------------------------------------
    vt = sbuf.tile([P, 16 * B], mm_dt)
    for t in range(16):
        pt = psum_t.tile([P, B], mm_dt, name="ptv", tag="pt")
        nc.tensor.transpose(pt[:], v[:, t * P : (t + 1) * P], ident[0:B, 0:B])
        nc.vector.tensor_copy(vt[:, t * B : (t + 1) * B], pt[:])

    # --- MM2: p = v @ WoT_k + bo  ([64, 256]) ----------------------------
    wo_sb = sbuf.tile([P, 16 * EK], mm_dt)  # d-tile t at cols [256t, 256t+256)
    nc.sync.dma_start(
        out=wo_sb[:].rearrange("p (t e) -> p t e", e=EK),
        in_=wot.rearrange("(t p) e -> p t e", p=P),
    )
    pp = psum_p.tile([P, P], F32, tag="pp")
    for half in range(2):
        o = pp[half * B : (half + 1) * B, :]
        for t in range(16):
            nc.tensor.matmul(
                o,
                vt[:, t * B : (t + 1) * B],
                wo_sb[:, t * EK + half * P : t * EK + (half + 1) * P],
                start=(t == 0),
                stop=False,
            )
        nc.tensor.matmul(
            o, ones1[:], bo_sb[:, half * P : (half + 1) * P], start=False, stop=True
        )
    # --- sequence + broadcast(p) -----------------------------------------
    # seq layout (host-prepared): partition p = 64*eh + b, free = 32*s' ... i.e.
    # seq_sb[64*eh + b, s*128 + e'] = sequence[b, s, ek + 128*eh + e'].
    # p broadcasts along the free s-dim (step-0), which DVE supports.
    seq_sb = sbuf.tile([P, 2 * B * S], F32)
    nc.sync.dma_start(out=seq_sb[:], in_=seq[:])
    p_re = sbuf.tile([P, P], F32)
    nc.vector.tensor_copy(p_re[:], pp[:])
    out_sb = sbuf.tile([P, 2 * B * S], F32)
    nc.vector.tensor_add(
        out_sb[:].rearrange("p (s e) -> p s e", e=P),
        seq_sb[:].rearrange("p (s e) -> p s e", e=P),
        p_re[:, None, :].to_broadcast((P, S, P)),
    )
    nc.sync.dma_start(out=out[:], in_=out_sb[:])
    ctx.close()


def _get_nc():
    if "nc" not in _CACHE:
        _CACHE["nc"] = _build_nc()
    return _CACHE["nc"]


def _shard(sequence, controls, Wv, bv, Wo, bo):
    wvt = np.ascontiguousarray(Wv.T)
    ctrl = np.ascontiguousarray(controls.reshape(C * B, D))
    in_maps = []
    for k in range(N_CORES):
        ek = slice(k * EK, (k + 1) * EK)
        in_maps.append(
            {
                "seq": np.ascontiguousarray(
                    sequence[:, :, ek]
                    .reshape(B, S, 2, 128)
                    .transpose(2, 0, 1, 3)
                    .reshape(128, S * 128)
                ),
                "ctrl": ctrl,
                "wvt": wvt,
                "wot": np.ascontiguousarray(Wo[ek, :].T),
                "bv": np.ascontiguousarray(bv),
                "bo": np.ascontiguousarray(bo[ek]),
            }
        )
    return in_maps


def _run(inputs, trace=False):
    nc = _get_nc()
    in_maps = _shard(
        inputs["sequence"], inputs["controls"], inputs["Wv"], inputs["bv"],
        inputs["Wo"], inputs["bo"],
    )
    res = run_bass_kernel_spmd(nc, in_maps, list(range(N_CORES)), trace=trace)
    out = np.empty((B, S, D), dtype=np.float32)
    for k in range(N_CORES):
        out[:, :, k * EK : (k + 1) * EK] = (
            res.results[k]["out"].reshape(2, B, S, 128).transpose(1, 2, 0, 3).reshape(B, S, EK)
        )
    return out, res


def kernel(**inputs):
    out, _ = _run(inputs)
    return out
